# revision 7
# baseline (speedup 1.0000x reference)
"""CogVLM vision attention on 8 trn2 NeuronCores.

Sharding: data-parallel over batch (B=8 -> one batch element per core).
Each core runs the full attention layer for its batch element; no collectives.

Per-core layout strategy (all matmul inputs bf16, fp32 PSUM accumulation):
  Y = X^T           [H=1792(part), S]   via PE transpose of X tiles
  Q^T_h, K^T_h      [112(part), S]      per head, lhsT = qkv_w column slice
  V                 [S(part), 16, 113]  lhsT = Y; per-head block has d0..95 at
                                        cols 0..95, ones at col 96 (softmax
                                        denominator trick), d96..111 at 97..112
  scores^T          [t(part), s]        lhsT = K^T_h[:, t-tile], rhs = Q^T_h
  expS = exp(SCALE*scores) (no max-subtraction: |scores| is O(5), safe in f32)
  attn_un^T + sums  [113(part), s]      lhsT = V[t-tile, h-block], rhs = expS;
                                        psum row 96 = softmax denominators
  normalize         recip(psum[96:97]) -> K=1 ones matmul broadcast -> DVE mult
  out               [s(part), o]        lhsT = ATT_h (K=113/head, 16 heads),
                                        rhs = permuted dense_w rows (+0 row 96)
Padding: S=1226 padded to 1280; t-pad masked by zeroed V pad rows.
"""
import math

import numpy as np

H = 1792
NH = 16
HD = 112
S = 1226
SP = 1280            # padded S
KT = H // 128        # 14
TT = 10              # t tiles of 128 (last has 74 valid rows)
LAST_T_ROWS = S - 9 * 128  # 74
SCALE = 1.0 / math.sqrt(HD)
S_MM = [(0, 512), (512, 512), (1024, 256)]            # free-dim s tiles
O_MM = [(0, 512), (512, 512), (1024, 512), (1536, 256)]  # dense out free tiles
VN = [(i * 448, 448) for i in range(4)]               # V proj free tiles (4 heads)

_CACHED = {}


def _build():
    import concourse.bass as bass
    import concourse.mybir as mybir
    from concourse import bacc
    from concourse.tile import TileContext
    from concourse.masks import make_identity

    F32 = mybir.dt.float32
    BF16 = mybir.dt.bfloat16
    Exp = mybir.ActivationFunctionType.Exp
    Ident = mybir.ActivationFunctionType.Identity
    MUL = mybir.AluOpType.mult

    nc = bacc.Bacc("TRN2", target_bir_lowering=False)

    x_d = nc.dram_tensor("x", [S, H], F32, kind="ExternalInput")
    qkvw_d = nc.dram_tensor("qkv_w", [H, 3 * H], F32, kind="ExternalInput")
    qkvb_d = nc.dram_tensor("qkv_b", [3 * H], F32, kind="ExternalInput")
    dw_d = nc.dram_tensor("dense_w", [H, H], F32, kind="ExternalInput")
    db_d = nc.dram_tensor("dense_b", [H], F32, kind="ExternalInput")
    out_d = nc.dram_tensor("out", [S, H], F32, kind="ExternalOutput")

    qkvw_r = qkvw_d.rearrange("(ko p) m -> p ko m", p=128)  # [128, 14, 5376]

    with TileContext(nc) as tc:
        with tc.tile_pool(name="const", bufs=1) as const:
            ident = const.tile([128, 128], F32, tag="ident")
            make_identity(nc, ident)
            ones1 = const.tile([1, 512], BF16, tag="ones1")
            nc.vector.memset(ones1[:], 1.0)
            vb16 = const.tile([1, H], BF16, tag="vb16")
            db16 = const.tile([1, H], BF16, tag="db16")
            qb16 = const.tile([1, 2 * H], BF16, tag="qb16")
            with tc.tile_pool(name="cstg", bufs=1) as cstg:
                vb_f = cstg.tile([1, H], F32, tag="vb_f")
                nc.sync.dma_start(vb_f[:], qkvb_d[None, 2 * H:])
                nc.vector.tensor_copy(vb16[:], vb_f[:])
                db_f = cstg.tile([1, H], F32, tag="db_f")
                nc.sync.dma_start(db_f[:], db_d[None, :])
                nc.vector.tensor_copy(db16[:], db_f[:])
                qb_f = cstg.tile([1, 2 * H], F32, tag="qb_f")
                nc.sync.dma_start(qb_f[:], qkvb_d[None, :2 * H])
                nc.vector.tensor_copy(qb16[:], qb_f[:])

            with tc.tile_pool(name="yv", bufs=1) as yv:
                Y = [yv.tile([128, SP], BF16, tag=f"Y{k}", name=f"Y{k}") for k in range(KT)]
                V = [yv.tile([128, NH, 113], BF16, tag=f"V{t}", name=f"V{t}") for t in range(TT)]

                # ---------------- Phase A: X^T and V projection ----------------
                with tc.tile_pool(name="pha", bufs=2) as pha, \
                     tc.tile_pool(name="wv16p", bufs=1) as wv16p, \
                     tc.tile_pool(name="tp_ps", bufs=3, space="PSUM") as tp_ps, \
                     tc.tile_pool(name="vp_ps", bufs=2, space="PSUM") as vp_ps:
                    WV = [wv16p.tile([128, H], BF16, tag=f"WV{k}", name=f"WV{k}") for k in range(KT)]
                    for ki in range(KT):
                        wv_f = pha.tile([128, H], F32, tag="wv_f")
                        nc.gpsimd.dma_start(wv_f[:], qkvw_r[:, ki, 2 * H:])
                        nc.vector.tensor_copy(WV[ki][:], wv_f[:])

                    for ti in range(TT):
                        rows = LAST_T_ROWS if ti == 9 else 128
                        x_t = pha.tile([128, H], F32, tag="x_t")
                        if ti == 9:
                            nc.vector.memset(x_t[:], 0.0)
                        nc.sync.dma_start(x_t[:rows, :], x_d[ti * 128:ti * 128 + rows, :])
                        for ki in range(KT):
                            tp = tp_ps.tile([128, 128], F32, tag="tp")
                            nc.tensor.transpose(
                                tp[:], x_t[:, ki * 128:(ki + 1) * 128], ident[:])
                            nc.vector.tensor_copy(
                                Y[ki][:, ti * 128:(ti + 1) * 128], tp[:])

                        # V projection for this t tile
                        if ti == 9:
                            nc.vector.memset(V[ti][:], 0.0)
                        for ni, (n0, nw) in enumerate(VN):
                            vps = vp_ps.tile([128, 448], F32, tag="vps")
                            for ki in range(KT):
                                nc.tensor.matmul(
                                    vps[:], Y[ki][:, ti * 128:(ti + 1) * 128],
                                    WV[ki][:, n0:n0 + nw],
                                    start=(ki == 0), stop=False)
                            nc.tensor.matmul(
                                vps[:], ones1[:, :128], vb16[:, n0:n0 + nw],
                                start=False, stop=True)
                            vv = vps[:rows].rearrange("p (h d) -> p h d", h=4)
                            nc.vector.tensor_copy(
                                V[ti][:rows, 4 * ni:4 * ni + 4, 0:96], vv[:, :, 0:96])
                            nc.vector.tensor_copy(
                                V[ti][:rows, 4 * ni:4 * ni + 4, 97:113], vv[:, :, 96:112])
                        nc.vector.memset(V[ti][:rows, :, 96:97], 1.0)

                # ---------------- Phase B: per-head attention ----------------
                with tc.tile_pool(name="att", bufs=1) as att:
                    ATT = [att.tile([128, SP], BF16, tag=f"ATT{h}", name=f"ATT{h}") for h in range(NH)]
                    with tc.tile_pool(name="qkwf", bufs=2) as qkwf, \
                         tc.tile_pool(name="qkw", bufs=4) as qkw, \
                         tc.tile_pool(name="qkt", bufs=4) as qkt, \
                         tc.tile_pool(name="esb", bufs=3) as esb, \
                         tc.tile_pool(name="small", bufs=4) as small, \
                         tc.tile_pool(name="pj_ps", bufs=2, space="PSUM") as pj_ps, \
                         tc.tile_pool(name="sc_ps", bufs=2, space="PSUM") as sc_ps, \
                         tc.tile_pool(name="at_ps", bufs=3, space="PSUM") as at_ps, \
                         tc.tile_pool(name="rb_ps", bufs=1, space="PSUM") as rb_ps:
                        for h in range(NH):
                            # project Q^T_h, K^T_h
                            qkT = []
                            for qk in range(2):
                                col0 = qk * H + h * HD
                                w_f = qkwf.tile([128, KT, HD], F32, tag="w_f")
                                nc.sync.dma_start(
                                    w_f[:], qkvw_r[:, :, col0:col0 + HD])
                                w16 = qkw.tile([128, KT, HD], BF16, tag="w16")
                                nc.vector.tensor_copy(w16[:], w_f[:])
                                pT = qkt.tile([128, SP], BF16, tag="pT")
                                for s0, sw in S_MM:
                                    pps = pj_ps.tile([128, 512], F32, tag="pps")
                                    for ki in range(KT):
                                        nc.tensor.matmul(
                                            pps[:HD, :sw], w16[:, ki, :],
                                            Y[ki][:, s0:s0 + sw],
                                            start=(ki == 0), stop=False)
                                    nc.tensor.matmul(
                                        pps[:HD, :sw],
                                        qb16[:, col0:col0 + HD],
                                        ones1[:, :sw], start=False, stop=True)
                                    nc.scalar.copy(
                                        pT[:HD, s0:s0 + sw], pps[:HD, :sw])
                                qkT.append(pT)
                            qT, kT = qkT

                            ats = []
                            for si in range(len(S_MM)):
                                aps = at_ps.tile([128, 512], F32, tag="aps",
                                                 name=f"aps{h}_{si}")
                                ats.append(aps)

                            for ti in range(TT):
                                es = esb.tile([128, SP], BF16, tag="es")
                                for si, (s0, sw) in enumerate(S_MM):
                                    scp = sc_ps.tile([128, 512], F32, tag="scp")
                                    nc.tensor.matmul(
                                        scp[:, :sw],
                                        kT[:HD, ti * 128:(ti + 1) * 128],
                                        qT[:HD, s0:s0 + sw],
                                        start=True, stop=True)
                                    nc.scalar.activation(
                                        es[:, s0:s0 + sw], scp[:, :sw], Exp,
                                        scale=float(SCALE))
                                for si, (s0, sw) in enumerate(S_MM):
                                    nc.tensor.matmul(
                                        ats[si][:113, :sw], V[ti][:, h, :],
                                        es[:, s0:s0 + sw],
                                        start=(ti == 0), stop=(ti == TT - 1))

                            # free attn psum banks fast: dump unnormalized
                            # attn + sums to SBUF, normalize in place later
                            sums = []
                            for si, (s0, sw) in enumerate(S_MM):
                                nc.vector.tensor_copy(
                                    ATT[h][:113, s0:s0 + sw], ats[si][:113, :sw])
                                s16 = small.tile([1, 512], F32, tag="s16",
                                                 name=f"s16_{h}_{si}")
                                nc.scalar.copy(s16[:, :sw], ats[si][96:97, :sw])
                                sums.append(s16)
                            for si, (s0, sw) in enumerate(S_MM):
                                recip = small.tile([1, 512], F32, tag="recip")
                                nc.vector.reciprocal(
                                    recip[:, :sw], sums[si][:, :sw])
                                recip16 = small.tile([1, 512], BF16, tag="recip16")
                                nc.vector.tensor_copy(recip16[:, :sw], recip[:, :sw])
                                rbp = rb_ps.tile([128, 512], F32, tag="rbp")
                                nc.tensor.matmul(
                                    rbp[:113, :sw], ones1[:, :113],
                                    recip16[:, :sw], start=True, stop=True)
                                rbs = small.tile([128, 512], BF16, tag="rbs")
                                nc.scalar.copy(rbs[:113, :sw], rbp[:113, :sw])
                                nc.vector.tensor_tensor(
                                    ATT[h][:113, s0:s0 + sw],
                                    ATT[h][:113, s0:s0 + sw],
                                    rbs[:113, :sw], MUL)

                    # ---------------- Phase C: dense projection ----------------
                    with tc.tile_pool(name="dwf", bufs=3) as dwfp, \
                         tc.tile_pool(name="osb", bufs=2) as osb, \
                         tc.tile_pool(name="dn_ps", bufs=4, space="PSUM") as dn_ps:
                        # dense weights reuse the (now dead) Y/V slots in the yv pool
                        DW = [yv.tile([128, H], BF16,
                                      tag=(f"Y{h}" if h < KT else f"V{h - KT}"),
                                      name=f"DW{h}") for h in range(NH)]
                        for h in range(NH):
                            dwf = dwfp.tile([128, H], F32, tag="dwf")
                            nc.vector.memset(dwf[96:97, :], 0.0)
                            nc.gpsimd.dma_start(
                                dwf[0:96, :], dw_d[h * HD:h * HD + 96, :])
                            nc.gpsimd.dma_start(
                                dwf[97:113, :], dw_d[h * HD + 96:h * HD + HD, :])
                            nc.vector.tensor_copy(DW[h][:113, :], dwf[:113, :])

                        for si in range(TT):
                            rows = LAST_T_ROWS if si == 9 else 128
                            for o0, ow in O_MM:
                                dps = dn_ps.tile([128, 512], F32, tag="dps")
                                for h in range(NH):
                                    nc.tensor.matmul(
                                        dps[:, :ow],
                                        ATT[h][:113, si * 128:(si + 1) * 128],
                                        DW[h][:113, o0:o0 + ow],
                                        start=(h == 0), stop=False)
                                nc.tensor.matmul(
                                    dps[:, :ow], ones1[:, :128],
                                    db16[:, o0:o0 + ow], start=False, stop=True)
                                ot = osb.tile([128, 512], F32, tag="ot")
                                nc.vector.tensor_copy(ot[:rows, :ow], dps[:rows, :ow])
                                nc.sync.dma_start(
                                    out_d[si * 128:si * 128 + rows, o0:o0 + ow],
                                    ot[:rows, :ow])
    nc.finalize()
    return nc


def get_nc():
    if "nc" not in _CACHED:
        _CACHED["nc"] = _build()
    return _CACHED["nc"]


def kernel(hidden_state, qkv_w, qkv_b, dense_w, dense_b, **run_kwargs):
    from concourse.bass_utils import run_bass_kernel_spmd

    nc = get_nc()
    B = hidden_state.shape[0]
    assert B == 8
    shared = {
        "qkv_w": np.ascontiguousarray(qkv_w, dtype=np.float32),
        "qkv_b": np.ascontiguousarray(qkv_b, dtype=np.float32),
        "dense_w": np.ascontiguousarray(dense_w, dtype=np.float32),
        "dense_b": np.ascontiguousarray(dense_b, dtype=np.float32),
    }
    in_maps = [
        {"x": np.ascontiguousarray(hidden_state[b], dtype=np.float32), **shared}
        for b in range(B)
    ]
    res = run_bass_kernel_spmd(nc, in_maps, core_ids=list(range(B)), **run_kwargs)
    out = np.stack([r["out"] for r in res.results])
    if run_kwargs:
        _CACHED["last_results"] = res
    return out


# revision 9
# speedup vs baseline: 1.1393x; 1.1393x over previous
"""CogVLM vision attention on 8 trn2 NeuronCores.

Sharding: data-parallel over batch (B=8 -> one batch element per core).
Each core runs the full attention layer for its batch element; no collectives.

Per-core layout strategy (all matmul inputs bf16, fp32 PSUM accumulation):
  Y = X^T           [H=1792(part), S]   via PE transpose of X tiles
  Q^T_h, K^T_h      [112(part), S]      per head, lhsT = qkv_w column slice
  V                 [S(part), 16, 113]  lhsT = Y; per-head block has d0..95 at
                                        cols 0..95, ones at col 96 (softmax
                                        denominator trick), d96..111 at 97..112
  scores^T          [t(part), s]        lhsT = K^T_h[:, t-tile], rhs = Q^T_h
  expS = exp(SCALE*scores) (no max-subtraction: |scores| is O(5), safe in f32)
  attn_un^T + sums  [113(part), s]      lhsT = V[t-tile, h-block], rhs = expS;
                                        psum row 96 = softmax denominators
  normalize         recip(psum[96:97]) -> K=1 ones matmul broadcast -> DVE mult
  out               [s(part), o]        lhsT = ATT_h (K=113/head, 16 heads),
                                        rhs = permuted dense_w rows (+0 row 96)
Padding: S=1226 padded to 1280; t-pad masked by zeroed V pad rows.
"""
import math

import numpy as np

H = 1792
NH = 16
HD = 112
S = 1226
SP = 1280            # padded S
KT = H // 128        # 14
TT = 10              # t tiles of 128 (last has 74 valid rows)
LAST_T_ROWS = S - 9 * 128  # 74
SCALE = 1.0 / math.sqrt(HD)
S_MM = [(0, 512), (512, 512), (1024, 256)]            # free-dim s tiles
O_MM = [(0, 512), (512, 512), (1024, 512), (1536, 256)]  # dense out free tiles
VN = [(i * 448, 448) for i in range(4)]               # V proj free tiles (4 heads)

_CACHED = {}


def _build():
    import concourse.bass as bass
    import concourse.mybir as mybir
    from concourse import bacc
    from concourse.tile import TileContext
    from concourse.masks import make_identity

    F32 = mybir.dt.float32
    BF16 = mybir.dt.bfloat16
    Exp = mybir.ActivationFunctionType.Exp
    Ident = mybir.ActivationFunctionType.Identity
    MUL = mybir.AluOpType.mult

    nc = bacc.Bacc("TRN2", target_bir_lowering=False)

    x_d = nc.dram_tensor("x", [S, H], F32, kind="ExternalInput")
    qkvw_d = nc.dram_tensor("qkv_w", [H, 3 * H], F32, kind="ExternalInput")
    qkvb_d = nc.dram_tensor("qkv_b", [3 * H], F32, kind="ExternalInput")
    dw_d = nc.dram_tensor("dense_w", [H, H], F32, kind="ExternalInput")
    db_d = nc.dram_tensor("dense_b", [H], F32, kind="ExternalInput")
    out_d = nc.dram_tensor("out", [S, H], F32, kind="ExternalOutput")

    qkvw_r = qkvw_d.rearrange("(ko p) m -> p ko m", p=128)  # [128, 14, 5376]

    with TileContext(nc) as tc:
        with tc.tile_pool(name="const", bufs=1) as const:
            ident = const.tile([128, 128], F32, tag="ident")
            make_identity(nc, ident)
            ones1 = const.tile([1, 512], BF16, tag="ones1")
            nc.vector.memset(ones1[:], 1.0)
            vb16 = const.tile([1, H], BF16, tag="vb16")
            db16 = const.tile([1, H], BF16, tag="db16")
            qb16 = const.tile([1, 2 * H], BF16, tag="qb16")
            with tc.tile_pool(name="cstg", bufs=1) as cstg:
                vb_f = cstg.tile([1, H], F32, tag="vb_f")
                nc.sync.dma_start(vb_f[:], qkvb_d[None, 2 * H:])
                nc.vector.tensor_copy(vb16[:], vb_f[:])
                db_f = cstg.tile([1, H], F32, tag="db_f")
                nc.sync.dma_start(db_f[:], db_d[None, :])
                nc.vector.tensor_copy(db16[:], db_f[:])
                qb_f = cstg.tile([1, 2 * H], F32, tag="qb_f")
                nc.sync.dma_start(qb_f[:], qkvb_d[None, :2 * H])
                nc.vector.tensor_copy(qb16[:], qb_f[:])

            with tc.tile_pool(name="yv", bufs=1) as yv:
                Y = [yv.tile([128, SP], BF16, tag=f"Y{k}", name=f"Y{k}") for k in range(KT)]
                V = [yv.tile([128, NH, 113], BF16, tag=f"V{t}", name=f"V{t}") for t in range(TT)]

                # ---------------- Phase A: X^T and V projection ----------------
                with tc.tile_pool(name="pha", bufs=2) as pha, \
                     tc.tile_pool(name="wv16p", bufs=1) as wv16p, \
                     tc.tile_pool(name="tp_ps", bufs=3, space="PSUM") as tp_ps, \
                     tc.tile_pool(name="vp_ps", bufs=2, space="PSUM") as vp_ps:
                    WV = [wv16p.tile([128, H], BF16, tag=f"WV{k}", name=f"WV{k}") for k in range(KT)]
                    for ki in range(KT):
                        wv_f = pha.tile([128, H], F32, tag="wv_f")
                        nc.gpsimd.dma_start(wv_f[:], qkvw_r[:, ki, 2 * H:])
                        nc.vector.tensor_copy(WV[ki][:], wv_f[:])

                    for ti in range(TT):
                        rows = LAST_T_ROWS if ti == 9 else 128
                        x_t = pha.tile([128, H], F32, tag="x_t")
                        if ti == 9:
                            nc.vector.memset(x_t[:], 0.0)
                        nc.sync.dma_start(x_t[:rows, :], x_d[ti * 128:ti * 128 + rows, :])
                        for ki in range(KT):
                            tp = tp_ps.tile([128, 128], F32, tag="tp")
                            nc.tensor.transpose(
                                tp[:], x_t[:, ki * 128:(ki + 1) * 128], ident[:])
                            nc.vector.tensor_copy(
                                Y[ki][:, ti * 128:(ti + 1) * 128], tp[:])

                        # V projection for this t tile
                        if ti == 9:
                            nc.vector.memset(V[ti][:], 0.0)
                        for ni, (n0, nw) in enumerate(VN):
                            vps = vp_ps.tile([128, 448], F32, tag="vps")
                            for ki in range(KT):
                                nc.tensor.matmul(
                                    vps[:], Y[ki][:, ti * 128:(ti + 1) * 128],
                                    WV[ki][:, n0:n0 + nw],
                                    start=(ki == 0), stop=False)
                            nc.tensor.matmul(
                                vps[:], ones1[:, :128], vb16[:, n0:n0 + nw],
                                start=False, stop=True)
                            vv = vps[:rows].rearrange("p (h d) -> p h d", h=4)
                            nc.vector.tensor_copy(
                                V[ti][:rows, 4 * ni:4 * ni + 4, 0:96], vv[:, :, 0:96])
                            nc.vector.tensor_copy(
                                V[ti][:rows, 4 * ni:4 * ni + 4, 97:113], vv[:, :, 96:112])
                        nc.vector.memset(V[ti][:rows, :, 96:97], 1.0)

                # ---------------- Phase B: per-head attention ----------------
                with tc.tile_pool(name="att", bufs=1) as att:
                    ATT = [att.tile([128, SP], BF16, tag=f"ATT{h}", name=f"ATT{h}") for h in range(NH)]
                    with tc.tile_pool(name="qkwf", bufs=2) as qkwf, \
                         tc.tile_pool(name="qkw", bufs=4) as qkw, \
                         tc.tile_pool(name="qkt", bufs=4) as qkt, \
                         tc.tile_pool(name="esb", bufs=3) as esb, \
                         tc.tile_pool(name="small", bufs=2) as small, \
                         tc.tile_pool(name="sumsp", bufs=8) as sumsp, \
                         tc.tile_pool(name="pj_ps", bufs=2, space="PSUM") as pj_ps, \
                         tc.tile_pool(name="sc_ps", bufs=2, space="PSUM") as sc_ps, \
                         tc.tile_pool(name="at_ps", bufs=3, space="PSUM") as at_ps, \
                         tc.tile_pool(name="rb_ps", bufs=1, space="PSUM") as rb_ps:
                        pending = []

                        def emit_normalize(hh, sums):
                            for si, (s0, sw) in enumerate(S_MM):
                                recip = small.tile([1, 512], F32, tag="recip")
                                nc.vector.reciprocal(
                                    recip[:, :sw], sums[si][:, :sw])
                                recip16 = small.tile([1, 512], BF16, tag="recip16")
                                nc.vector.tensor_copy(recip16[:, :sw], recip[:, :sw])
                                rbp = rb_ps.tile([128, 512], F32, tag="rbp")
                                nc.tensor.matmul(
                                    rbp[:113, :sw], ones1[:, :113],
                                    recip16[:, :sw], start=True, stop=True)
                                rbs = small.tile([128, 512], BF16, tag="rbs")
                                nc.scalar.copy(rbs[:113, :sw], rbp[:113, :sw])
                                nc.vector.tensor_tensor(
                                    ATT[hh][:113, s0:s0 + sw],
                                    ATT[hh][:113, s0:s0 + sw],
                                    rbs[:113, :sw], MUL)

                        for h in range(NH):
                            # project Q^T_h, K^T_h
                            qkT = []
                            for qk in range(2):
                                col0 = qk * H + h * HD
                                w_f = qkwf.tile([128, KT, HD], F32, tag="w_f")
                                nc.sync.dma_start(
                                    w_f[:], qkvw_r[:, :, col0:col0 + HD])
                                w16 = qkw.tile([128, KT, HD], BF16, tag="w16")
                                nc.vector.tensor_copy(w16[:], w_f[:])
                                pT = qkt.tile([128, SP], BF16, tag="pT")
                                for s0, sw in S_MM:
                                    pps = pj_ps.tile([128, 512], F32, tag="pps")
                                    for ki in range(KT):
                                        nc.tensor.matmul(
                                            pps[:HD, :sw], w16[:, ki, :],
                                            Y[ki][:, s0:s0 + sw],
                                            start=(ki == 0), stop=False)
                                    nc.tensor.matmul(
                                        pps[:HD, :sw],
                                        qb16[:, col0:col0 + HD],
                                        ones1[:, :sw], start=False, stop=True)
                                    nc.scalar.copy(
                                        pT[:HD, s0:s0 + sw], pps[:HD, :sw])
                                qkT.append(pT)
                            qT, kT = qkT

                            ats = []
                            for si in range(len(S_MM)):
                                aps = at_ps.tile([128, 512], F32, tag="aps",
                                                 name=f"aps{h}_{si}")
                                ats.append(aps)

                            for ti in range(TT):
                                es = esb.tile([128, SP], BF16, tag="es")
                                for si, (s0, sw) in enumerate(S_MM):
                                    scp = sc_ps.tile([128, 512], F32, tag="scp")
                                    nc.tensor.matmul(
                                        scp[:, :sw],
                                        kT[:HD, ti * 128:(ti + 1) * 128],
                                        qT[:HD, s0:s0 + sw],
                                        start=True, stop=True)
                                    nc.scalar.activation(
                                        es[:, s0:s0 + sw], scp[:, :sw], Exp,
                                        scale=float(SCALE))
                                for si, (s0, sw) in enumerate(S_MM):
                                    nc.tensor.matmul(
                                        ats[si][:113, :sw], V[ti][:, h, :],
                                        es[:, s0:s0 + sw],
                                        start=(ti == 0), stop=(ti == TT - 1))

                            # free attn psum banks fast: dump unnormalized
                            # attn + sums to SBUF; normalization is deferred by
                            # one head so the reciprocal chain (serial ~10us on
                            # DVE) never blocks the PE instruction stream
                            sums = []
                            for si, (s0, sw) in enumerate(S_MM):
                                nc.vector.tensor_copy(
                                    ATT[h][:113, s0:s0 + sw], ats[si][:113, :sw])
                                s16 = sumsp.tile([1, 512], F32, tag="s16",
                                                 name=f"s16_{h}_{si}")
                                nc.scalar.copy(s16[:, :sw], ats[si][96:97, :sw])
                                sums.append(s16)
                            pending.append((h, sums))
                            if len(pending) > 1:
                                emit_normalize(*pending.pop(0))

                        for hh, sums in pending:
                            emit_normalize(hh, sums)

                    # ---------------- Phase C: dense projection ----------------
                    with tc.tile_pool(name="dwf", bufs=3) as dwfp, \
                         tc.tile_pool(name="osb", bufs=2) as osb, \
                         tc.tile_pool(name="dn_ps", bufs=4, space="PSUM") as dn_ps:
                        # dense weights reuse the (now dead) Y/V slots in the yv pool
                        DW = [yv.tile([128, H], BF16,
                                      tag=(f"Y{h}" if h < KT else f"V{h - KT}"),
                                      name=f"DW{h}") for h in range(NH)]
                        for h in range(NH):
                            dwf = dwfp.tile([128, H], F32, tag="dwf")
                            nc.vector.memset(dwf[96:97, :], 0.0)
                            nc.gpsimd.dma_start(
                                dwf[0:96, :], dw_d[h * HD:h * HD + 96, :])
                            nc.gpsimd.dma_start(
                                dwf[97:113, :], dw_d[h * HD + 96:h * HD + HD, :])
                            nc.vector.tensor_copy(DW[h][:113, :], dwf[:113, :])

                        for si in range(TT):
                            rows = LAST_T_ROWS if si == 9 else 128
                            for o0, ow in O_MM:
                                dps = dn_ps.tile([128, 512], F32, tag="dps")
                                for h in range(NH):
                                    nc.tensor.matmul(
                                        dps[:, :ow],
                                        ATT[h][:113, si * 128:(si + 1) * 128],
                                        DW[h][:113, o0:o0 + ow],
                                        start=(h == 0), stop=False)
                                nc.tensor.matmul(
                                    dps[:, :ow], ones1[:, :128],
                                    db16[:, o0:o0 + ow], start=False, stop=True)
                                ot = osb.tile([128, 512], F32, tag="ot")
                                nc.vector.tensor_copy(ot[:rows, :ow], dps[:rows, :ow])
                                nc.sync.dma_start(
                                    out_d[si * 128:si * 128 + rows, o0:o0 + ow],
                                    ot[:rows, :ow])
    nc.finalize()
    return nc


def get_nc():
    if "nc" not in _CACHED:
        _CACHED["nc"] = _build()
    return _CACHED["nc"]


def kernel(hidden_state, qkv_w, qkv_b, dense_w, dense_b, **run_kwargs):
    from concourse.bass_utils import run_bass_kernel_spmd

    nc = get_nc()
    B = hidden_state.shape[0]
    assert B == 8
    shared = {
        "qkv_w": np.ascontiguousarray(qkv_w, dtype=np.float32),
        "qkv_b": np.ascontiguousarray(qkv_b, dtype=np.float32),
        "dense_w": np.ascontiguousarray(dense_w, dtype=np.float32),
        "dense_b": np.ascontiguousarray(dense_b, dtype=np.float32),
    }
    in_maps = [
        {"x": np.ascontiguousarray(hidden_state[b], dtype=np.float32), **shared}
        for b in range(B)
    ]
    res = run_bass_kernel_spmd(nc, in_maps, core_ids=list(range(B)), **run_kwargs)
    out = np.stack([r["out"] for r in res.results])
    if run_kwargs:
        _CACHED["last_results"] = res
    return out


# revision 10
# speedup vs baseline: 1.1582x; 1.0166x over previous
"""CogVLM vision attention on 8 trn2 NeuronCores.

Sharding: data-parallel over batch (B=8 -> one batch element per core).
Each core runs the full attention layer for its batch element; no collectives.

Per-core layout strategy (all matmul inputs bf16, fp32 PSUM accumulation):
  Y = X^T           [H=1792(part), S]   via PE transpose of X tiles
  Q^T_h, K^T_h      [112(part), S]      per head, lhsT = qkv_w column slice
  V                 [S(part), 16, 113]  lhsT = Y; per-head block has d0..95 at
                                        cols 0..95, ones at col 96 (softmax
                                        denominator trick), d96..111 at 97..112
  scores^T          [t(part), s]        lhsT = K^T_h[:, t-tile], rhs = Q^T_h
  expS = exp(SCALE*scores) (no max-subtraction: |scores| is O(5), safe in f32)
  attn_un^T + sums  [113(part), s]      lhsT = V[t-tile, h-block], rhs = expS;
                                        psum row 96 = softmax denominators
  normalize         recip(psum[96:97]) -> K=1 ones matmul broadcast -> DVE mult
  out               [s(part), o]        lhsT = ATT_h (K=113/head, 16 heads),
                                        rhs = permuted dense_w rows (+0 row 96)
Padding: S=1226 padded to 1280; t-pad masked by zeroed V pad rows.
"""
import math

import numpy as np

H = 1792
NH = 16
HD = 112
S = 1226
SP = 1280            # padded S
KT = H // 128        # 14
TT = 10              # t tiles of 128 (last has 74 valid rows)
LAST_T_ROWS = S - 9 * 128  # 74
SCALE = 1.0 / math.sqrt(HD)
S_MM = [(0, 512), (512, 512), (1024, 202)]            # free-dim s tiles (202: stop at S=1226)
O_MM = [(0, 512), (512, 512), (1024, 512), (1536, 256)]  # dense out free tiles
VN = [(i * 448, 448) for i in range(4)]               # V proj free tiles (4 heads)

_CACHED = {}


def _build():
    import concourse.bass as bass
    import concourse.mybir as mybir
    from concourse import bacc
    from concourse.tile import TileContext
    from concourse.masks import make_identity

    F32 = mybir.dt.float32
    BF16 = mybir.dt.bfloat16
    Exp = mybir.ActivationFunctionType.Exp
    Ident = mybir.ActivationFunctionType.Identity
    MUL = mybir.AluOpType.mult

    nc = bacc.Bacc("TRN2", target_bir_lowering=False)

    x_d = nc.dram_tensor("x", [S, H], F32, kind="ExternalInput")
    qkvw_d = nc.dram_tensor("qkv_w", [H, 3 * H], F32, kind="ExternalInput")
    qkvb_d = nc.dram_tensor("qkv_b", [3 * H], F32, kind="ExternalInput")
    dw_d = nc.dram_tensor("dense_w", [H, H], F32, kind="ExternalInput")
    db_d = nc.dram_tensor("dense_b", [H], F32, kind="ExternalInput")
    out_d = nc.dram_tensor("out", [S, H], F32, kind="ExternalOutput")

    qkvw_r = qkvw_d.rearrange("(ko p) m -> p ko m", p=128)  # [128, 14, 5376]

    with TileContext(nc) as tc:
        with tc.tile_pool(name="const", bufs=1) as const:
            ident16 = const.tile([128, 128], BF16, tag="ident16")
            make_identity(nc, ident16)
            ones1 = const.tile([1, 512], BF16, tag="ones1")
            nc.vector.memset(ones1[:], 1.0)
            vb16 = const.tile([1, H], BF16, tag="vb16")
            db16 = const.tile([1, H], BF16, tag="db16")
            qb16 = const.tile([1, 2 * H], BF16, tag="qb16")
            with tc.tile_pool(name="cstg", bufs=1) as cstg:
                vb_f = cstg.tile([1, H], F32, tag="vb_f")
                nc.sync.dma_start(vb_f[:], qkvb_d[None, 2 * H:])
                nc.vector.tensor_copy(vb16[:], vb_f[:])
                db_f = cstg.tile([1, H], F32, tag="db_f")
                nc.sync.dma_start(db_f[:], db_d[None, :])
                nc.vector.tensor_copy(db16[:], db_f[:])
                qb_f = cstg.tile([1, 2 * H], F32, tag="qb_f")
                nc.sync.dma_start(qb_f[:], qkvb_d[None, :2 * H])
                nc.vector.tensor_copy(qb16[:], qb_f[:])

            with tc.tile_pool(name="yv", bufs=1) as yv:
                Y = [yv.tile([128, SP], BF16, tag=f"Y{k}", name=f"Y{k}") for k in range(KT)]
                V = [yv.tile([128, NH, 113], BF16, tag=f"V{t}", name=f"V{t}") for t in range(TT)]

                # ---------------- Phase A: X^T and V projection ----------------
                with tc.tile_pool(name="pha", bufs=2) as pha, \
                     tc.tile_pool(name="wv16p", bufs=1) as wv16p, \
                     tc.tile_pool(name="tp_ps", bufs=3, space="PSUM") as tp_ps, \
                     tc.tile_pool(name="vp_ps", bufs=2, space="PSUM") as vp_ps:
                    WV = [wv16p.tile([128, H], BF16, tag=f"WV{k}", name=f"WV{k}") for k in range(KT)]
                    for ki in range(KT):
                        wv_f = pha.tile([128, H], F32, tag="wv_f")
                        nc.gpsimd.dma_start(wv_f[:], qkvw_r[:, ki, 2 * H:])
                        nc.vector.tensor_copy(WV[ki][:], wv_f[:])

                    for ti in range(TT):
                        rows = LAST_T_ROWS if ti == 9 else 128
                        x_t = pha.tile([128, H], F32, tag="x_t")
                        if ti == 9:
                            nc.vector.memset(x_t[:], 0.0)
                        nc.sync.dma_start(x_t[:rows, :], x_d[ti * 128:ti * 128 + rows, :])
                        x16 = pha.tile([128, H], BF16, tag="x16")
                        nc.vector.tensor_copy(x16[:], x_t[:])
                        for ki in range(KT):
                            tp = tp_ps.tile([128, 128], BF16, tag="tp")
                            nc.tensor.transpose(
                                tp[:], x16[:, ki * 128:(ki + 1) * 128], ident16[:])
                            nc.vector.tensor_copy(
                                Y[ki][:, ti * 128:(ti + 1) * 128], tp[:])

                        # V projection for this t tile
                        if ti == 9:
                            nc.vector.memset(V[ti][:], 0.0)
                        for ni, (n0, nw) in enumerate(VN):
                            vps = vp_ps.tile([128, 448], F32, tag="vps")
                            for ki in range(KT):
                                nc.tensor.matmul(
                                    vps[:], Y[ki][:, ti * 128:(ti + 1) * 128],
                                    WV[ki][:, n0:n0 + nw],
                                    start=(ki == 0), stop=False)
                            nc.tensor.matmul(
                                vps[:], ones1[:, :128], vb16[:, n0:n0 + nw],
                                start=False, stop=True)
                            vv = vps[:rows].rearrange("p (h d) -> p h d", h=4)
                            nc.vector.tensor_copy(
                                V[ti][:rows, 4 * ni:4 * ni + 4, 0:96], vv[:, :, 0:96])
                            nc.vector.tensor_copy(
                                V[ti][:rows, 4 * ni:4 * ni + 4, 97:113], vv[:, :, 96:112])
                        nc.vector.memset(V[ti][:rows, :, 96:97], 1.0)

                # ---------------- Phase B: per-head attention ----------------
                with tc.tile_pool(name="att", bufs=1) as att:
                    ATT = [att.tile([128, SP], BF16, tag=f"ATT{h}", name=f"ATT{h}") for h in range(NH)]
                    with tc.tile_pool(name="qkwf", bufs=2) as qkwf, \
                         tc.tile_pool(name="qkw", bufs=4) as qkw, \
                         tc.tile_pool(name="qkt", bufs=4) as qkt, \
                         tc.tile_pool(name="esb", bufs=3) as esb, \
                         tc.tile_pool(name="small", bufs=2) as small, \
                         tc.tile_pool(name="sumsp", bufs=8) as sumsp, \
                         tc.tile_pool(name="pj_ps", bufs=2, space="PSUM") as pj_ps, \
                         tc.tile_pool(name="sc_ps", bufs=2, space="PSUM") as sc_ps, \
                         tc.tile_pool(name="at_ps", bufs=3, space="PSUM") as at_ps, \
                         tc.tile_pool(name="rb_ps", bufs=1, space="PSUM") as rb_ps:
                        pending = []

                        def emit_normalize(hh, sums):
                            for si, (s0, sw) in enumerate(S_MM):
                                recip = small.tile([1, 512], F32, tag="recip")
                                nc.vector.reciprocal(
                                    recip[:, :sw], sums[si][:, :sw])
                                recip16 = small.tile([1, 512], BF16, tag="recip16")
                                nc.vector.tensor_copy(recip16[:, :sw], recip[:, :sw])
                                rbp = rb_ps.tile([128, 512], F32, tag="rbp")
                                nc.tensor.matmul(
                                    rbp[:113, :sw], ones1[:, :113],
                                    recip16[:, :sw], start=True, stop=True)
                                rbs = small.tile([128, 512], BF16, tag="rbs")
                                nc.scalar.copy(rbs[:113, :sw], rbp[:113, :sw])
                                nc.vector.tensor_tensor(
                                    ATT[hh][:113, s0:s0 + sw],
                                    ATT[hh][:113, s0:s0 + sw],
                                    rbs[:113, :sw], MUL)

                        for h in range(NH):
                            # project Q^T_h, K^T_h
                            qkT = []
                            for qk in range(2):
                                col0 = qk * H + h * HD
                                w_f = qkwf.tile([128, KT, HD], F32, tag="w_f")
                                nc.sync.dma_start(
                                    w_f[:], qkvw_r[:, :, col0:col0 + HD])
                                w16 = qkw.tile([128, KT, HD], BF16, tag="w16")
                                nc.vector.tensor_copy(w16[:], w_f[:])
                                pT = qkt.tile([128, SP], BF16, tag="pT")
                                for s0, sw in S_MM:
                                    pps = pj_ps.tile([128, 512], F32, tag="pps")
                                    for ki in range(KT):
                                        nc.tensor.matmul(
                                            pps[:HD, :sw], w16[:, ki, :],
                                            Y[ki][:, s0:s0 + sw],
                                            start=(ki == 0), stop=False)
                                    nc.tensor.matmul(
                                        pps[:HD, :sw],
                                        qb16[:, col0:col0 + HD],
                                        ones1[:, :sw], start=False, stop=True)
                                    nc.scalar.copy(
                                        pT[:HD, s0:s0 + sw], pps[:HD, :sw])
                                qkT.append(pT)
                            qT, kT = qkT

                            ats = []
                            for si in range(len(S_MM)):
                                aps = at_ps.tile([128, 512], F32, tag="aps",
                                                 name=f"aps{h}_{si}")
                                ats.append(aps)

                            for ti in range(TT):
                                es = esb.tile([128, SP], BF16, tag="es")
                                for si, (s0, sw) in enumerate(S_MM):
                                    scp = sc_ps.tile([128, 512], F32, tag="scp")
                                    nc.tensor.matmul(
                                        scp[:, :sw],
                                        kT[:HD, ti * 128:(ti + 1) * 128],
                                        qT[:HD, s0:s0 + sw],
                                        start=True, stop=True)
                                    nc.scalar.activation(
                                        es[:, s0:s0 + sw], scp[:, :sw], Exp,
                                        scale=float(SCALE))
                                for si, (s0, sw) in enumerate(S_MM):
                                    nc.tensor.matmul(
                                        ats[si][:113, :sw], V[ti][:, h, :],
                                        es[:, s0:s0 + sw],
                                        start=(ti == 0), stop=(ti == TT - 1))

                            # free attn psum banks fast: dump unnormalized
                            # attn + sums to SBUF; normalization is deferred by
                            # one head so the reciprocal chain (serial ~10us on
                            # DVE) never blocks the PE instruction stream
                            sums = []
                            for si, (s0, sw) in enumerate(S_MM):
                                nc.vector.tensor_copy(
                                    ATT[h][:113, s0:s0 + sw], ats[si][:113, :sw])
                                s16 = sumsp.tile([1, 512], F32, tag="s16",
                                                 name=f"s16_{h}_{si}")
                                nc.scalar.copy(s16[:, :sw], ats[si][96:97, :sw])
                                sums.append(s16)
                            pending.append((h, sums))
                            if len(pending) > 1:
                                emit_normalize(*pending.pop(0))

                        for hh, sums in pending:
                            emit_normalize(hh, sums)

                    # ---------------- Phase C: dense projection ----------------
                    with tc.tile_pool(name="dwf", bufs=3) as dwfp, \
                         tc.tile_pool(name="osb", bufs=2) as osb, \
                         tc.tile_pool(name="dn_ps", bufs=4, space="PSUM") as dn_ps:
                        # dense weights reuse the (now dead) Y/V slots in the yv pool
                        DW = [yv.tile([128, H], BF16,
                                      tag=(f"Y{h}" if h < KT else f"V{h - KT}"),
                                      name=f"DW{h}") for h in range(NH)]
                        for h in range(NH):
                            dwf = dwfp.tile([128, H], F32, tag="dwf")
                            nc.vector.memset(dwf[96:97, :], 0.0)
                            nc.gpsimd.dma_start(
                                dwf[0:96, :], dw_d[h * HD:h * HD + 96, :])
                            nc.gpsimd.dma_start(
                                dwf[97:113, :], dw_d[h * HD + 96:h * HD + HD, :])
                            nc.vector.tensor_copy(DW[h][:113, :], dwf[:113, :])

                        for si in range(TT):
                            rows = LAST_T_ROWS if si == 9 else 128
                            for o0, ow in O_MM:
                                dps = dn_ps.tile([128, 512], F32, tag="dps")
                                for h in range(NH):
                                    nc.tensor.matmul(
                                        dps[:, :ow],
                                        ATT[h][:113, si * 128:(si + 1) * 128],
                                        DW[h][:113, o0:o0 + ow],
                                        start=(h == 0), stop=False)
                                nc.tensor.matmul(
                                    dps[:, :ow], ones1[:, :128],
                                    db16[:, o0:o0 + ow], start=False, stop=True)
                                ot = osb.tile([128, 512], F32, tag="ot")
                                nc.vector.tensor_copy(ot[:rows, :ow], dps[:rows, :ow])
                                nc.sync.dma_start(
                                    out_d[si * 128:si * 128 + rows, o0:o0 + ow],
                                    ot[:rows, :ow])
    nc.finalize()
    return nc


def get_nc():
    if "nc" not in _CACHED:
        _CACHED["nc"] = _build()
    return _CACHED["nc"]


def kernel(hidden_state, qkv_w, qkv_b, dense_w, dense_b, **run_kwargs):
    from concourse.bass_utils import run_bass_kernel_spmd

    nc = get_nc()
    B = hidden_state.shape[0]
    assert B == 8
    shared = {
        "qkv_w": np.ascontiguousarray(qkv_w, dtype=np.float32),
        "qkv_b": np.ascontiguousarray(qkv_b, dtype=np.float32),
        "dense_w": np.ascontiguousarray(dense_w, dtype=np.float32),
        "dense_b": np.ascontiguousarray(dense_b, dtype=np.float32),
    }
    in_maps = [
        {"x": np.ascontiguousarray(hidden_state[b], dtype=np.float32), **shared}
        for b in range(B)
    ]
    res = run_bass_kernel_spmd(nc, in_maps, core_ids=list(range(B)), **run_kwargs)
    out = np.stack([r["out"] for r in res.results])
    if run_kwargs:
        _CACHED["last_results"] = res
    return out
